# revision 49
# baseline (speedup 1.0000x reference)
"""AreaSelfAttention Trainium2 kernel (8 NeuronCores, pure data parallel).

Reference computation (per full input):
  pad x [4,256,252,252] -> [4,256,256,256]; 1x1 convs q,k (256->32), v (256->256);
  8x8 windows; attn = softmax(q^T k) over j; out = v @ attn^T; unwindow;
  final = gamma * out + x  (crop back to 252x252).

Design (v2):
  - Shard over (batch, wrow-half): each core gets 16 "wrows" of 2048
    window-major pixels (32 windows). x ships ONCE per core as fp8 e4m3 in
    [128, 2(c-half), pix] layout, DMA'd in 4-wrow chunks (16KB/partition
    descriptors). Device output is UNNORMALIZED PV plus a rowsum carrier
    column, bf16 [wrow-pair, 128, 2, 16, 257], DMA'd per wrow-pair.
  - Host finishes: out = x + gamma*bv + gamma*(oT'/rowsum) in f32 (softmax
    affinity: attn(v+bv) = attn(v)+bv, so no bias/residual ships).
  - K-bias dropped (softmax-invariant); Q-bias rides the q evac.
  - All convs use fp8 DoubleRow matmuls (K=256 contracted in one
    instruction at 0.5 cycles/row): qk conv = 8 matmuls/wrow emitting q
    and k into separate psum banks at partition bases {0,32,64,96} (one
    512-px block per base), evac'd straight into the sT operand layout --
    no SBUF gather DMAs. vT conv = 16 DoubleRow matmuls/wrow (128-pixel
    pair-stationary), carrier column 256 memset to 1.0.
  - sT: one [32,128]x[32,128]->[128,128] matmul per window pair (16/wrow);
    the off-diagonal cross-window blocks are computed garbage, exp'd, then
    memset to 0 (where PV's K=128 pair contraction needs zeros).
  - PV: per pair K=128 N=257 bf16 matmul (eT stationary, vt moving with
    rowsum carrier), evac per 2 pairs.
  - Evacs are spread across ACT/DVE/GPSIMD per a static assignment table.
  - Software pipeline: x prefetched one 4-wrow chunk ahead; wrow g-1's PV
    emitted between wrow g's conv phase and sT phase.
"""

from contextlib import ExitStack

import numpy as np
import ml_dtypes

import bass_rust as br
import concourse.bass as bass
import concourse.tile as tile
from concourse import mybir
from concourse.bass_utils import run_bass_kernel_spmd

FP32 = mybir.dt.float32
BF16 = mybir.dt.bfloat16
F8 = mybir.dt.float8e4
AF = mybir.ActivationFunctionType
DR = mybir.MatmulPerfMode.DoubleRow

B, C, H, W = 4, 256, 252, 252
A = 8
PH = PW = 256
NH = NW = 32
CR = 32
NCORES = 8
G = 16          # wrows per core
PIX = 2048      # pixels per wrow (32 windows * 64)


def _split_wide_waits(nc, max_waits=1):
    """walrus on this toolchain rejects >1 sync wait per instruction; move
    excess waits onto preceding same-engine NoOps (equivalent semantics)."""
    n = 0
    for fn in nc.m.functions:
        for bb in fn.blocks:
            insts = list(bb.instructions)
            new, changed = [], False
            for inst in insts:
                si = inst.sync_info
                waits = list(si.on_wait) if si is not None else []
                if len(waits) > max_waits:
                    changed = True
                    chunks = [waits[i:i + max_waits]
                              for i in range(0, len(waits), max_waits)]
                    for ch in chunks[:-1]:
                        nop = br.InstNoOp(name=f"I-wsplit-{n}", ins=[], outs=[])
                        n += 1
                        nop.engine = inst.engine
                        nop.sync_info = br.SyncInfo(on_wait=ch, on_update=[])
                        new.append(nop)
                    inst.sync_info = br.SyncInfo(
                        on_wait=chunks[-1], on_update=list(si.on_update))
                new.append(inst)
            if changed:
                bb.instructions = new
    return n


def build_nc():
    nc = bass.Bass()
    x_d = nc.declare_dram_parameter("x", [G, 128, 2, PIX], F8,
                                    isOutput=False)
    # all weights/bias packed into one byte tensor -> a single const DMA
    # (per-queue DGE config is ~1us serial on SP; 4 queues cost ~4us of
    # startup before the x(0) transfer can begin)
    # layout per partition c: [wqk bytes 0:128 | wvt bytes 128:640 |
    #                          bq4 f32 bytes 640:644]
    cst_d = nc.declare_dram_parameter("cst", [128, 644], F8, isOutput=False)
    out_d = nc.declare_dram_parameter("out", [G, 128, 16, C + 1], BF16,
                                      isOutput=True)

    with tile.TileContext(nc) as tc, ExitStack() as ctx:
        consts = ctx.enter_context(tc.tile_pool(name="consts", bufs=1))
        xbp = ctx.enter_context(tc.tile_pool(name="xbp", bufs=3))
        qk0p = ctx.enter_context(tc.tile_pool(name="qk0p", bufs=2))
        etp = ctx.enter_context(tc.tile_pool(name="etp", bufs=2))
        vtp = ctx.enter_context(tc.tile_pool(name="vtp", bufs=2))
        otp = ctx.enter_context(tc.tile_pool(name="otp", bufs=2))

        stqk_ps = ctx.enter_context(
            tc.tile_pool(name="stqk_ps", bufs=2, space="PSUM"))
        vt_ps = ctx.enter_context(
            tc.tile_pool(name="vt_ps", bufs=2, space="PSUM"))
        pv_ps = ctx.enter_context(
            tc.tile_pool(name="pv_ps", bufs=2, space="PSUM"))

        def load_x(g, split=False):
            x8 = xbp.tile([128, 2, PIX], F8, tag="x8", name=f"x8_{g}")
            if split:
                # per-half DMAs let the first h=0 matmuls start while the
                # h=1 half is still in flight (startup latency)
                for h in range(2):
                    nc.sync.dma_start(out=x8[:, h, :], in_=x_d[g, :, h, :])
            else:
                nc.sync.dma_start(out=x8, in_=x_d[g])
            return x8

        x0_early = load_x(0, split=True)

        cst_b = consts.tile([128, 644], F8, tag="cst")
        nc.sync.dma_start(out=cst_b, in_=cst_d[:])
        _cb = cst_b[:, :]

        def _cst(off, ap):
            return bass.AP(tensor=_cb.tensor, offset=_cb.offset + off, ap=ap)

        # wqk[:, h, c0:c0+32] equivalent: [128, 32] at byte h*64+c0
        def wqk_sl(h, c0):
            return _cst(h * 64 + c0, [[644, 128], [1, 32]])

        wvt_b = _cst(128, [[644, 128], [256, 2], [1, 256]])  # [128, 2, 256]
        bq4_b = cst_b[:, 640:644].bitcast(FP32)              # [128, 1] f32

        # evac engine helpers -------------------------------------------------
        def evac(eng, dst, src):
            if eng == "a":
                nc.scalar.activation(out=dst, in_=src, func=AF.Copy)
            elif eng == "v":
                nc.vector.tensor_copy(out=dst, in_=src)
            else:
                nc.gpsimd.tensor_copy(out=dst, in_=src)

        # evac engine tables: ACT: q + 4 exp + 3 vt + 3 pv;
        #                     DVE: k + 5 vt + 5 pv
        vt_engine = ["a", "v", "a", "v", "v", "a", "v", "v"]
        pv_engine = ["v", "a", "v", "a", "v", "a", "v", "v"]

        def emit_pv_group(gp, q2, state, oT_p):
            eT_p, vt_p = state
            pv2 = pv_ps.tile([128, 2, 512], FP32, tag="pv")
            for pi in range(2):
                p = q2 * 2 + pi
                sg, ec = p // 4, (p % 4) * 128
                nc.tensor.matmul(pv2[:, pi, 0:257],
                                 eT_p[:, sg, ec:ec + 128],
                                 vt_p[:, p, :], start=True, stop=True)
            dst = oT_p[:, 2 * q2:2 * q2 + 2, :]
            evac(pv_engine[q2], dst, pv2[:, :, 0:257])
            if q2 == 3:
                nc.sync.dma_start(out=out_d[gp, :, 0:8, :], in_=oT_p[:, 0:8, :])
            elif q2 == 7:
                nc.sync.dma_start(out=out_d[gp, :, 8:16, :],
                                  in_=oT_p[:, 8:16, :])

        def emit_wrow(g, x8, state, last=False):
            """One wrow's conv/score work, interleaved with wrow g-1's PV."""
            # qk conv: normal fp8 matmuls col-tiled at {0,32,64,96} (s3d3/
            # DoubleRow demands dst base 0, unusable here); the 4 strips
            # run concurrently in distinct PE sub-arrays, the c-half
            # accumulation pairs serialize per strip.
            qps = stqk_ps.tile([128, 512], FP32, tag="st")
            kps = stqk_ps.tile([128, 512], FP32, tag="st")
            for ps, c0 in ((qps, 0), (kps, 32)):
                for h in range(2):
                    for b4 in range(4):
                        sa = slice(b4 * 512, (b4 + 1) * 512)
                        nc.tensor.matmul(ps[32 * b4:32 * b4 + 32, :],
                                         wqk_sl(h, c0), x8[:, h, sa],
                                         start=(h == 0), stop=(h == 1),
                                         skip_group_check=True,
                                         tile_position=(0, 32 * b4))
            q0 = qk0p.tile([128, 512], BF16, tag="q0", name=f"q0_{g}")
            k0 = qk0p.tile([128, 512], BF16, tag="k0", name=f"k0_{g}")
            nc.scalar.add(q0, qps, bq4_b)
            nc.vector.tensor_copy(out=k0, in_=kps)

            oT_p = None
            if state is not None:
                oT_p = otp.tile([128, 16, C + 1], BF16, tag="oT",
                                name=f"oT_{g - 1}")

            # vT conv into vt[128, 16, 257] bf16 (col 256 = 1.0 carrier);
            # pair p = 128 px: x-block stationary, DoubleRow over c-halves.
            # Interleave: vt group i, then PV group i of wrow g-1, then
            # (for i>=4) sT group i-4 -- smooths the evac-engine streams.
            vt_g = vtp.tile([128, 16, 257], BF16, tag="vt", name=f"vt_{g}")
            nc.gpsimd.memset(vt_g[:, :, 256:257], 1.0)
            eT_g = etp.tile([128, 4, 512], BF16, tag="eT", name=f"eT_{g}")
            oT_own = None
            for i in range(8):
                vps = vt_ps.tile([128, 2, 256], FP32, tag="vtps")
                for j in range(2):
                    p0 = i * 256 + j * 128
                    nc.tensor.matmul(vps[:, j, :], x8[:, :, p0:p0 + 128],
                                     wvt_b, perf_mode=DR,
                                     skip_group_check=True)
                evac(vt_engine[i], vt_g[:, 2 * i:2 * i + 2, 0:256], vps)
                if state is not None:
                    emit_pv_group(g - 1, i, state, oT_p)
                # sT pair matmuls for blocks (sg, sg+1) back to back: the
                # two groups sit in adjacent PE row-strips (32sg, 32sg+32)
                # and overlap. One [32,128]x[32,128] matmul per pair writes
                # the full [128,128] block (off-diagonal cross-window
                # garbage exp'd then zeroed).
                sgs = ()
                if last and i in (0, 2):
                    sgs = (i, i + 1)
                elif not last and i in (4, 6):
                    sgs = (i - 4, i - 3)
                sps_l = []
                for sg in sgs:
                    sps = stqk_ps.tile([128, 512], FP32, tag="st")
                    sps_l.append(sps)
                    pb = slice(32 * sg, 32 * sg + 32)
                    for pl in range(4):
                        cw = slice(pl * 128, (pl + 1) * 128)
                        nc.tensor.matmul(sps[:, cw], k0[pb, cw], q0[pb, cw],
                                         skip_group_check=True,
                                         tile_position=(32 * sg, 0))
                for sg, sps in zip(sgs, sps_l):
                    nc.scalar.activation(out=eT_g[:, sg, :], in_=sps,
                                         func=AF.Exp)
                    top = eT_g[0:64, sg, :]
                    nc.gpsimd.memset(
                        bass.AP(tensor=top.tensor, offset=top.offset + 64,
                                ap=[[2048, 64], [128, 4], [1, 64]]), 0.0)
                    bot = eT_g[64:128, sg, :]
                    nc.gpsimd.memset(
                        bass.AP(tensor=bot.tensor, offset=bot.offset,
                                ap=[[2048, 64], [128, 4], [1, 64]]), 0.0)
                if last and i >= 5:
                    # start this (final) wrow's own PV as soon as its eT/vt
                    # groups land, to shorten the pipeline drain
                    if oT_own is None:
                        oT_own = otp.tile([128, 16, C + 1], BF16, tag="oT",
                                          name=f"oT_{g}")
                    emit_pv_group(g, i - 5, (eT_g, vt_g), oT_own)
            return eT_g, vt_g, oT_own

        prev = None
        xq = {0: x0_early, 1: load_x(1)}
        oT_own = None
        for g in range(G):
            eT_g, vt_g, oT_own = emit_wrow(g, xq.pop(g), prev,
                                           last=(g == G - 1))
            if g + 2 < G:
                xq[g + 2] = load_x(g + 2)
            prev = (eT_g, vt_g)
        # flush the remaining PV groups of the final wrow
        for q2 in range(3, 8):
            emit_pv_group(G - 1, q2, prev, oT_own)

    _split_wide_waits(nc)
    return nc


_NC_CACHE = None


def _get_nc():
    global _NC_CACHE
    if _NC_CACHE is None:
        _NC_CACHE = build_nc()
    return _NC_CACHE


def _prep_inputs(x, Wq, bq, Wk, bk, Wv, bv, gamma):
    """Host-side: pad + window-major permute + shard x; pack weights."""
    xp = np.zeros((B, C, PH, PW), np.float32)
    xp[:, :, :H, :W] = x
    # window-major: [b, c, nh, nw, r, wc] -> [b, c, wrow, pix]
    xw = xp.reshape(B, C, NH, A, NW, A).transpose(0, 1, 2, 4, 3, 5)
    xw = np.ascontiguousarray(xw).reshape(B, C, NH, PIX)

    shards = []
    for core in range(NCORES):
        b, hr = core // 2, core % 2
        sh = xw[b, :, hr * G:(hr + 1) * G, :]            # [256, G, PIX]
        sh = sh.reshape(2, 128, G, PIX).transpose(2, 1, 0, 3)  # [G,128,2,PIX]
        shards.append(np.ascontiguousarray(sh).astype(ml_dtypes.float8_e4m3))

    wqk = np.concatenate([Wq.T, Wk.T], axis=1)          # [256, 64]
    wqk = wqk.reshape(2, 128, 64).transpose(1, 0, 2)    # [c, pair, 64]
    wqk = np.ascontiguousarray(wqk).astype(ml_dtypes.float8_e4m3)
    wqk = wqk.reshape(128, 128)
    wvt = Wv.T.reshape(2, 128, 256).astype(ml_dtypes.float8_e4m3)  # [in, out]
    wvt = np.concatenate([wvt[0], wvt[1]], axis=1)      # [128, 512]
    bq4 = np.tile(bq, 4).reshape(128, 1).astype(np.float32)
    f8 = ml_dtypes.float8_e4m3
    cst = np.concatenate([wqk.view(np.uint8), wvt.view(np.uint8),
                          bq4.view(np.uint8)], axis=1).view(f8)  # [128, 644]
    cst = np.ascontiguousarray(cst)

    in_maps = []
    for core in range(NCORES):
        in_maps.append({
            "x": shards[core],
            "cst": cst,
        })
    return in_maps


def _gather_output(results, x, bv, gamma):
    raw = np.stack([results[i]["out"].astype(np.float32)
                    for i in range(NCORES)])  # [8, G, 128, 16, C+1]
    attn = (raw[..., 0:C] / raw[..., C:C + 1]
            * np.float32(gamma[0]))  # normalize by rowsum carrier
    attn = attn.reshape(B, 2 * G, 128, 16, C).transpose(0, 1, 3, 2, 4)
    attn = attn.reshape(B, 2 * G, PIX, C).transpose(0, 3, 1, 2)  # [b,c,nh,pix]
    attn = attn.reshape(B, C, NH, NW, A, A).transpose(0, 1, 2, 4, 3, 5)
    attn = np.ascontiguousarray(attn).reshape(B, C, PH, PW)[:, :, :H, :W]
    gbv = (gamma.astype(np.float64)[0]
           * bv.astype(np.float64)).astype(np.float32)
    return x + gbv[None, :, None, None] + attn


def run(inputs, trace=False):
    nc = _get_nc()
    in_maps = _prep_inputs(**inputs)
    res = run_bass_kernel_spmd(nc, in_maps, core_ids=list(range(NCORES)),
                               trace=trace)
    out = _gather_output(res.results, np.asarray(inputs["x"], np.float32),
                        inputs["bv"], inputs["gamma"])
    return out, res


def kernel(**inputs):
    inputs = {k: np.asarray(v) for k, v in inputs.items()}
    out, _ = run(inputs)
    return out
